# revision 17
# baseline (speedup 1.0000x reference)
"""Bundle-adjustment loss kernel for 8 Trainium2 NeuronCores.

Data-parallel over the image axis M: each core processes exactly
12500 = 4*3125 images (no padding).

Device layout v2: partition dim = (quarter, camera) = 4*32 = 128 (all
partitions busy), free dim = (point, m') = 3*3125 = 9375.  The camera
transform R@P+t is done on the PE as 9 small matmuls per 512-block
(weights [13,128] encode delta(q,q')*R[c,i,j] plus a ones-row for t),
which also broadcasts the per-image point planes across cameras --- so
the DMA ships the compact [13,3125] point planes (0.24 MB) instead of
32x-replicated ones (7.4 MB).  The distortion polynomial + pixel
residual chain runs on wide [128, 3125] point-slices, emitted
slice-major with a fixed DVE/Pool/Act stage assignment (the "full"
row of ASSIGN below) tuned by interleaved A/B measurement: engine
handoffs inside a chain are expensive on this NRT, in-order engine
streams overlap only across slice boundaries, and DVE's 2x/4x modes
do not show up in practice.  Per-(q,c) camera scalars ride in scalar
slots of tensor_scalar / scalar_tensor_tensor / activation, so no
constant planes exist.  Observations arrive with the principal point
pre-subtracted (host) and the per-image losses accumulate on-chip via
activation(Sqrt, accum).  ~160-220 us/pass per core measured via the
reps-slope method (see test.py); v1 of this kernel was ~520 us.
"""

import numpy as np

M_TOTAL = 100000
C = 32
NCORES = 8
MCORE = 12500        # images per core
Q = 4                # image quarters -> partition = (q, c)
MQ = MCORE // Q      # 3125 images per quarter (free dim per point)
W = 3 * MQ           # 9375 total free width
BLK = 512            # matmul moving-dim block (one PSUM bank of f32)
KDIM = 13            # matmul contraction: 12 (q,j) rows + ones row
W_LOSS = 0.01        # LINE_W = LEN_W = REPROJ_W
ESCALE = 64.0        # e = (du^2+dv^2)/ESCALE; host multiplies by sqrt(ESCALE)
XNA = 100            # line/len images per partition row (125 rows used)

_NC_CACHE = {}


def _apply_tile_patch():
    """This walrus build rejects Tile's kernel-tail drain carrying every
    semaphore wait on one instruction ("Too many sync wait commands").
    Emit one wait_ge per live semaphore instead."""
    from concourse import tile

    if getattr(tile.TileContext, "_ba_drain_patched", False):
        return

    def _drain_and_barrier(self, tick_clock, wait_clock):
        nc = self.nc
        ticks = list(tick_clock.global_clock)
        allocated = wait_clock.sems.allocated()
        for key, sem in allocated.items():
            t = ticks[int(key)]
            if t > 0:
                nc.sync.wait_ge(sem, t)
        nc.sync.drain()
        nc.all_engine_barrier()
        assert self.sems is not None
        popped = nc._tile_sem_poison_stack.pop()
        assert popped is self._sem_poison
        nc.clear_and_free_semaphores(list(self.sems.allocated().values()))
        nc.all_engine_barrier()

    tile.TileContext._drain_and_barrier = _drain_and_barrier
    tile.TileContext._ba_drain_patched = True


def _spill_excess_waits(nc, cap=1):
    """This walrus build's ISA structs accept very few sync-wait slots
    per compute instruction. Spill waits beyond `cap` onto InstNoOp
    carriers inserted just before the instruction on the same engine."""
    import concourse.mybir as mybir
    import bass_rust

    fragile = {
        "InstTensorScalarPtr", "InstTensorScalar", "InstActivation",
        "InstReciprocal", "InstTensorReduce", "InstMatmult",
        "InstTensorCopy", "InstTensorTensor", "InstLdweights",
        "InstMemset", "InstIota", "InstTensorTensorReduce", "InstPool",
        "InstDMACopy", "InstDMA", "InstDmaTransposeAnt",
    }
    n_nop = 0
    for bb in nc.m.functions[0].blocks:
        il = bb.instructions
        out_list = []
        for inst in il:
            si = inst.sync_info
            if (si is not None and type(inst).__name__ in fragile
                    and len(si.on_wait) > cap):
                waits = list(si.on_wait)
                keep, spill = waits[:cap], waits[cap:]
                for wv in spill:
                    nop = mybir.InstNoOp(name=f"ba_waitnop_{n_nop}")
                    n_nop += 1
                    nop.engine = inst.engine
                    nop.sync_info = bass_rust.SyncInfo(
                        on_wait=[wv], on_update=[])
                    out_list.append(nop)
                inst.sync_info = bass_rust.SyncInfo(
                    on_wait=keep, on_update=list(si.on_update))
            out_list.append(inst)
        if len(out_list) != len(il):
            bb.instructions = out_list
    return n_nop


def _ap_key(arg):
    try:
        return str(arg)
    except Exception:
        return repr(arg)


def _dedup_ldweights(nc):
    """Drop InstLdweights whose payload equals the previous ldweights in
    the same block (PE array state is unchanged by intervening matmuls).
    Non-empty sync moves onto an InstNoOp carrier on the same engine."""
    import concourse.mybir as mybir

    n_drop = 0
    for bb in nc.m.functions[0].blocks:
        last = None
        keep = []
        for inst in bb.instructions:
            if type(inst).__name__ == "InstLdweights":
                key = tuple(_ap_key(a) for a in inst.ins)
                if last is not None and key == last:
                    si = inst.sync_info
                    if si is not None and (si.on_wait or si.on_update):
                        nop = mybir.InstNoOp(name=f"ldw_drop_{n_drop}")
                        nop.engine = inst.engine
                        nop.sync_info = si
                        keep.append(nop)
                    n_drop += 1
                    continue
                last = key
            keep.append(inst)
        if n_drop:
            bb.instructions = keep
    return n_drop


def _build_nc(a_coef, b_coef, s_len, reps=1, variant="full"):
    """Build the SPMD Bass module (same program on all 8 cores)."""
    key = (a_coef, b_coef, s_len, reps, variant)
    if key in _NC_CACHE:
        return _NC_CACHE[key]
    import concourse.bass as bass
    import concourse.mybir as mybir
    from concourse import tile

    _apply_tile_patch()
    F32 = mybir.dt.float32
    F16 = mybir.dt.float16
    F8 = mybir.dt.float8e4
    ALU = mybir.AluOpType
    ACT = mybir.ActivationFunctionType

    nc = bass.Bass(trn_type="TRN2")
    # inputs (per core)
    obs_u = nc.declare_dram_parameter("obs_u", [128, W], F16, isOutput=False)
    obs_v = nc.declare_dram_parameter("obs_v", [128, W], F16, isOutput=False)
    maskf = nc.declare_dram_parameter("maskf", [128, W], F16, isOutput=False)
    pj = [nc.declare_dram_parameter(f"pj{p}", [KDIM, MQ], F16,
                                    isOutput=False) for p in range(3)]
    wgt = nc.declare_dram_parameter("wgt", [KDIM, 384], F16, isOutput=False)
    xn = nc.declare_dram_parameter("xn", [128, 9 * XNA], F16, isOutput=False)
    # per-(q,c)-partition camera scalars, one column each:
    # 0:k1 1:k2 2:k3 3:2p1 4:2p2 5:fx*p2 6:fy*p1 7:fx 8:fy 9:-s(len bias)
    cscal = nc.declare_dram_parameter("cscal", [128, 12], F32, isOutput=False)
    out = nc.declare_dram_parameter("out", [256], F32, isOutput=True)

    # matmul m'-blocks: pairs of 512 -> one [128,1024] psum tile + 53 tail
    full_pairs = [(0, 1024), (1024, 1024), (2048, 1024)]
    tail = (3072, MQ - 3072)   # 53

    flags = set(variant.split("_"))
    nwin = 3
    for f in flags:
        if f and f[0] == "w" and f[1:].isdigit():
            nwin = int(f[1:])
    use_pm = "pm" in flags
    psa_bufs = 6 if use_pm else 3

    with tile.TileContext(nc) as tc:
        with tc.tile_pool(name="sb", bufs=1) as sb, \
             tc.tile_pool(name="psA", space="PSUM", bufs=psa_bufs) as ppA, \
             tc.tile_pool(name="psB", space="PSUM", bufs=2) as ppB:
            # --- resident tiles ---
            cs_t = sb.tile([128, 12], F32, tag="cs", name="cs_t")
            w_t = sb.tile([KDIM, 384], F16, tag="wg", name="w_t")
            pj_t = [sb.tile([KDIM, MQ], F16, tag=f"pj{p}", name=f"pj{p}_t")
                    for p in range(3)]
            ou_t = sb.tile([128, W], F16, tag="ou", name="ou_t")
            ov_t = sb.tile([128, W], F16, tag="ov", name="ov_t")
            mk_t = sb.tile([128, W], F16, tag="mk", name="mk_t")
            T0 = sb.tile([128, W], F16, tag="T0", name="T0")
            T1 = sb.tile([128, W], F16, tag="T1", name="T1")
            T2 = sb.tile([128, W], F16, tag="T2", name="T2")
            T3 = sb.tile([128, W], F16, tag="T3", name="T3")
            T4 = sb.tile([128, W], F16, tag="T4", name="T4")
            # line/len working tiles
            xn_t = sb.tile([128, 9 * XNA], F16, tag="xn", name="xn_t")
            g_t = sb.tile([128, 3 * XNA], F16, tag="g", name="g_t")
            dc_t = sb.tile([128, 6 * XNA], F16, tag="dc", name="dc_t")
            sq_t = sb.tile([128, 6 * XNA], F16, tag="sq", name="sq_t")
            rd_t = sb.tile([128, 2 * XNA], F32, tag="rd", name="rd_t")
            rt_t = sb.tile([128, 2 * XNA], F16, tag="rt", name="rt_t")
            ln_t = sb.tile([128, XNA], F16, tag="ln", name="ln_t")
            cb_t = sb.tile([128, XNA], F16, tag="cb", name="cb_t")
            # stages
            pt_stage = sb.tile([128, 8], F32, tag="pts", name="pt_stage")
            ll_stage = sb.tile([128, 1], F32, tag="lls", name="ll_stage")
            ptred = sb.tile([128, 1], F32, tag="ptr", name="ptred")
            # camera-scalar column APs
            k1s = cs_t[:, 0:1]
            k2s = cs_t[:, 1:2]
            k3s = cs_t[:, 2:3]
            tp1s = cs_t[:, 3:4]
            tp2s = cs_t[:, 4:5]
            fxp2s = cs_t[:, 5:6]
            fyp1s = cs_t[:, 6:7]
            fxs = cs_t[:, 7:8]
            fys = cs_t[:, 8:9]
            negs = cs_t[:, 9:10]

            # prologue loads (outside reps)
            nc.sync.dma_start(cs_t[:], cscal[:])
            nc.sync.dma_start(w_t[:], wgt[:])

            for _rep in range(reps):
                nc.sync.dma_start(xn_t[:], xn[:])
                for p in range(3):
                    nc.sync.dma_start(pj_t[p][:], pj[p][:])

                if variant == "dmaonly":
                    nc.sync.dma_start(ou_t[:], obs_u[:])
                    nc.scalar.dma_start(ov_t[:], obs_v[:])
                    nc.scalar.dma_start(mk_t[:], maskf[:])
                    nc.vector.tensor_reduce(
                        pt_stage[:, 0:1], ou_t[:],
                        mybir.AxisListType.X, ALU.add)
                    nc.vector.tensor_reduce(
                        ll_stage[:], xn_t[:], mybir.AxisListType.X, ALU.add)
                    continue

                # ---- line/len losses ----
                xnv = xn_t[:].rearrange("p (a j) -> p a j", a=XNA)
                x0s, x1s, x2s = xnv[:, :, 0:3], xnv[:, :, 3:6], xnv[:, :, 6:9]
                gv = g_t[:].rearrange("p (a j) -> p a j", a=XNA)
                dcv = dc_t[:].rearrange("p (a k j) -> p a k j", a=XNA, k=2)
                nc.vector.scalar_tensor_tensor(
                    gv, x0s, a_coef, x1s, ALU.mult, ALU.subtract)
                nc.vector.scalar_tensor_tensor(
                    dcv[:, :, 0, :], x2s, b_coef, gv, ALU.mult, ALU.add)
                nc.vector.scalar_tensor_tensor(
                    dcv[:, :, 1, :], x0s, 1.0, x2s, ALU.mult, ALU.subtract)
                nc.vector.scalar_tensor_tensor(
                    sq_t[:], dc_t[:], 1.0, dc_t[:], ALU.mult, ALU.mult)
                nc.vector.tensor_reduce(
                    rd_t[:].rearrange("p (a k) -> p a k", k=2),
                    sq_t[:].rearrange("p (a k j) -> p a k j", a=XNA, k=2),
                    mybir.AxisListType.X, ALU.add)
                nc.scalar.activation(rt_t[:], rd_t[:], ACT.Sqrt)
                rtv = rt_t[:].rearrange("p (a k) -> p a k", k=2)
                nc.scalar.activation(ln_t[:], rtv[:, :, 1], ACT.Abs,
                                     bias=negs)
                nc.gpsimd.tensor_tensor(
                    cb_t[:], rtv[:, :, 0], ln_t[:], ALU.add)
                nc.vector.tensor_reduce(
                    ll_stage[:], cb_t[:], mybir.AxisListType.X, ALU.add)

                # ---- reprojection loss ----
                vr2 = "vr2" in flags
                me = "me" in flags

                # chain windows over the full free width
                wb = [W * k // nwin for k in range(nwin + 1)]
                wins = [slice(wb[k], wb[k + 1]) for k in range(nwin)]

                # obs/mask loads, split across the SP and Act queues
                for cols in wins:
                    nc.sync.dma_start(ou_t[:, cols], obs_u[:, cols])
                    nc.scalar.dma_start(ov_t[:, cols], obs_v[:, cols])
                    nc.scalar.dma_start(mk_t[:, cols], maskf[:, cols])

                # camera transform on PE: for each coord i,
                # T_i[(q,c), m'] = sum_j R[c,i,j] P[q,p,j,m'] + t[c,i]
                dst = (T0, T1, T2)
                if not use_pm:
                    for p in range(3):
                        c0 = p * MQ
                        for i in range(3):
                            wap = w_t[:, 128 * i:128 * (i + 1)]
                            for b0, bw in full_pairs:
                                ps = ppA.tile([128, 1024], F32, tag="psA",
                                              name=f"ps_{_rep}_{p}_{i}_{b0}")
                                nc.tensor.matmul(
                                    ps[:, 0:512], wap,
                                    pj_t[p][:, b0:b0 + 512],
                                    start=True, stop=True)
                                nc.tensor.matmul(
                                    ps[:, 512:1024], wap,
                                    pj_t[p][:, b0 + 512:b0 + 1024],
                                    start=True, stop=True)
                                nc.scalar.activation(
                                    dst[i][:, c0 + b0:c0 + b0 + bw],
                                    ps[:], ACT.Copy)
                            b0, bw = tail
                            ps = ppB.tile([128, 64], F32, tag="psB",
                                          name=f"pst_{_rep}_{p}_{i}")
                            nc.tensor.matmul(
                                ps[:, 0:bw], wap, pj_t[p][:, b0:b0 + bw],
                                start=True, stop=True)
                            nc.scalar.activation(
                                dst[i][:, c0 + b0:c0 + b0 + bw],
                                ps[:, 0:bw], ACT.Copy)
                else:
                    # psum-direct: recip reads z from PSUM; x0n/x1n consume
                    # x,y from PSUM on Pool; no PSUM->SBUF copies at all.
                    blocks = [(0, 512), (512, 512), (1024, 512),
                              (1536, 512), (2048, 512), (2560, 512),
                              (3072, MQ - 3072)]
                    for p in range(3):
                        c0 = p * MQ
                        for b0, bw in blocks:
                            pxyz = []
                            for i in range(3):
                                wap = w_t[:, 128 * i:128 * (i + 1)]
                                ps = ppA.tile([128, 512], F32, tag="psA",
                                              name=f"ps_{_rep}_{p}_{i}_{b0}")
                                nc.tensor.matmul(
                                    ps[:, 0:bw], wap,
                                    pj_t[p][:, b0:b0 + bw],
                                    start=True, stop=True)
                                pxyz.append(ps)
                            bc = slice(c0 + b0, c0 + b0 + bw)
                            with nc.allow_low_precision(
                                    reason="1/Z in f16: Z~5, rel err "
                                           "~5e-4 vs 2e-2 budget"):
                                nc.vector.reciprocal(
                                    T3[:, bc], pxyz[2][:, 0:bw])
                            nc.vector.scalar_tensor_tensor(  # x0n
                                T0[:, bc], pxyz[0][:, 0:bw], 1.0,
                                T3[:, bc], ALU.mult, ALU.mult)
                            nc.vector.scalar_tensor_tensor(  # x1n
                                T1[:, bc], pxyz[1][:, 0:bw], 1.0,
                                T3[:, bc], ALU.mult, ALU.mult)

                # distortion + residual chain per window (slice-major:
                # each window's chain is consecutive in every engine's
                # stream; engines overlap across window boundaries).
                # Engine assignment per stage ("V"=DVE, "P"=Pool, "A"=Act)
                # is a 22-char string; chain order is fixed by the math.
                ASSIGN = {
                    "full": "VPPVVPAVVVVVVVVVVAAPPA",
                    "cl":   "VPPVVVVVVVVVVVVVVAAPPA",
                    "clp":  "VPPPPVVVVVVVVVVVVAAPPA",
                    "cla":  "VPPVVPVVVVVVVVVVVAAPPA",
                }
                akey = next((f for f in flags if f in ASSIGN), "full")
                asg = ASSIGN[akey]

                def _stt(e, dst, in0, sc, in1, op0, op1):
                    if e == "P":
                        assert (isinstance(sc, float) and sc == 1.0)
                        nc.gpsimd.tensor_tensor(dst, in0, in1, op1)
                    else:
                        nc.vector.scalar_tensor_tensor(
                            dst, in0, sc, in1, op0, op1)

                for wi, cols in enumerate(wins):
                    t0 = T0[:, cols]
                    t1 = T1[:, cols]
                    t2 = T2[:, cols]
                    t3 = T3[:, cols]
                    t4 = T4[:, cols]
                    ou = ou_t[:, cols]
                    ov = ov_t[:, cols]
                    mk = mk_t[:, cols]
                    if not use_pm:
                        with nc.allow_low_precision(
                                reason="1/Z in f16: Z~5, rel err ~5e-4 "
                                       "vs 2e-2 budget"):
                            nc.vector.reciprocal(t3, t2)            # 0
                        _stt(asg[1], t0, t0, 1.0, t3,
                             ALU.mult, ALU.mult)                    # 1 x0n
                        _stt(asg[2], t1, t1, 1.0, t3,
                             ALU.mult, ALU.mult)                    # 2 x1n
                    _stt(asg[3], t2, t0, 1.0, t0,
                         ALU.mult, ALU.mult)                        # 3 x0n^2
                    _stt(asg[4], t3, t1, 1.0, t1,
                         ALU.mult, ALU.mult)                        # 4 x1n^2
                    _stt(asg[5], t2, t2, 1.0, t3,
                         ALU.mult, ALU.add)                         # 5 r2
                    if asg[6] == "A":                               # 6 k3r2+k2
                        nc.scalar.activation(t3, t2, ACT.Identity,
                                             bias=k2s, scale=k3s)
                    else:
                        nc.vector.tensor_scalar(
                            t3, t2, k3s, k2s, ALU.mult, ALU.add)
                    _stt(asg[7], t3, t3, 1.0, t2,
                         ALU.mult, ALU.mult)                        # 7 *r2
                    nc.vector.scalar_tensor_tensor(                 # 8 +k1,*r2
                        t3, t3, k1s, t2, ALU.add, ALU.mult)
                    nc.vector.scalar_tensor_tensor(                 # 9 tan0
                        t4, t1, tp1s, t3, ALU.mult, ALU.add)
                    nc.vector.scalar_tensor_tensor(                 # 10 ra
                        t4, t0, tp2s, t4, ALU.mult, ALU.add)
                    nc.vector.scalar_tensor_tensor(                 # 11 mu
                        t3, t4, 1.0, t0, ALU.add, ALU.mult)
                    nc.vector.scalar_tensor_tensor(                 # 12 mv
                        t0, t4, 1.0, t1, ALU.add, ALU.mult)
                    nc.vector.scalar_tensor_tensor(                 # 13
                        t1, t3, fxs, ou, ALU.mult, ALU.subtract)
                    nc.vector.scalar_tensor_tensor(                 # 14 du
                        t1, t2, fxp2s, t1, ALU.mult, ALU.add)
                    nc.vector.scalar_tensor_tensor(                 # 15
                        t4, t0, fys, ov, ALU.mult, ALU.subtract)
                    nc.vector.scalar_tensor_tensor(                 # 16 dv
                        t4, t2, fyp1s, t4, ALU.mult, ALU.add)
                    nc.scalar.activation(t0, t1, ACT.Square,        # 17
                                         scale=0.125)
                    nc.scalar.activation(t3, t4, ACT.Square,        # 18
                                         scale=0.125)
                    _stt(asg[19], t0, t0, 1.0, t3,
                         ALU.mult, ALU.add)                         # 19 e
                    _stt(asg[20], t0, t0, 1.0, mk,
                         ALU.mult, ALU.mult)                        # 20 mask
                    nc.scalar.activation(t4, t0, ACT.Sqrt,          # 21 sum
                                         accum_out=pt_stage[:, wi:wi + 1])

            # ---- epilogue ----
            nc.vector.tensor_reduce(ptred[:], pt_stage[:, 0:nwin],
                                    mybir.AxisListType.X, ALU.add)
            nc.sync.dma_start(out[0:128], ptred[:])
            nc.sync.dma_start(out[128:256], ll_stage[:])

    _dedup_ldweights(nc)
    _spill_excess_waits(nc)
    _NC_CACHE[key] = nc
    return nc


def kernel(pole, pole_3ds, pole_2ds, mask, K, dist, R, t):
    pole = np.asarray(pole, np.float32)
    pole_3ds = np.asarray(pole_3ds, np.float32)
    pole_2ds = np.asarray(pole_2ds, np.float32)
    mask = np.asarray(mask)
    K = np.asarray(K, np.float32)
    dist = np.asarray(dist, np.float32)
    R = np.asarray(R, np.float32)
    t = np.asarray(t, np.float32)

    s = float(pole[0] + pole[1])
    a_coef = float(pole[1] / s)   # coefficient of X0 in exp_p1
    b_coef = float(pole[0] / s)   # coefficient of X2

    def rep(v):  # [C] -> [(q,c)] = tile over quarters
        return np.tile(v.astype(np.float32), Q)

    cscal = np.zeros((128, 12), np.float32)
    cscal[:, 0] = rep(dist[:, 0])                 # k1
    cscal[:, 1] = rep(dist[:, 1])                 # k2
    cscal[:, 2] = rep(dist[:, 4])                 # k3
    cscal[:, 3] = rep(2.0 * dist[:, 2])           # 2*p1
    cscal[:, 4] = rep(2.0 * dist[:, 3])           # 2*p2
    cscal[:, 5] = rep(K[:, 0, 0] * dist[:, 3])    # fx*p2
    cscal[:, 6] = rep(K[:, 1, 1] * dist[:, 2])    # fy*p1
    cscal[:, 7] = rep(K[:, 0, 0])                 # fx
    cscal[:, 8] = rep(K[:, 1, 1])                 # fy
    cscal[:, 9] = -s                              # len-loss bias

    # matmul weights: wgt[q*3+j, i*128 + q*32+c] = R[c,i,j];
    # ones-row 12 -> t[c,i]
    wgt = np.zeros((KDIM, 384), np.float16)
    for i in range(3):
        for q in range(Q):
            blockc = i * 128 + q * 32
            for j in range(3):
                wgt[q * 3 + j, blockc:blockc + 32] = R[:, i, j]
            wgt[12, blockc:blockc + 32] = t[:, i]

    u0_c = K[:, 0, 2]   # [C]
    v0_c = K[:, 1, 2]

    in_maps = []
    for core in range(NCORES):
        m0 = core * MCORE
        # obs planes [128, 9375]: row=q*32+c, col=p*MQ+m'
        p2 = pole_2ds[m0:m0 + MCORE].reshape(Q, MQ, C, 3, 2)
        ou = np.ascontiguousarray(
            (p2[..., 0] - u0_c[None, None, :, None])
            .transpose(0, 2, 3, 1)          # [q, c, p, m']
            .reshape(128, W).astype(np.float16))
        ov = np.ascontiguousarray(
            (p2[..., 1] - v0_c[None, None, :, None])
            .transpose(0, 2, 3, 1)
            .reshape(128, W).astype(np.float16))
        mkq = mask[m0:m0 + MCORE].reshape(Q, MQ, C).transpose(0, 2, 1)
        mk = np.ascontiguousarray(
            np.broadcast_to(mkq[:, :, None, :], (Q, C, 3, MQ))
            .reshape(128, W).astype(np.float16))
        # compact point planes pj[p][q*3+j, m'] = P[m, p, j]; row 12 = 1
        p3 = pole_3ds[m0:m0 + MCORE].reshape(Q, MQ, 3, 3)
        pjs = []
        for p in range(3):
            pl = np.zeros((KDIM, MQ), np.float16)
            pl[0:12] = p3[:, :, p, :].transpose(0, 2, 1).reshape(12, MQ)
            pl[12] = 1.0
            pjs.append(pl)
        # line/len layout: image m_local = r*XNA + a -> xn[r, 9a:9a+9]
        xnat = np.zeros((128, XNA, 9), np.float16)
        xnat[:125] = pole_3ds[m0:m0 + MCORE].reshape(125, XNA, 9)
        in_maps.append({
            "obs_u": ou, "obs_v": ov, "maskf": mk,
            "pj0": pjs[0], "pj1": pjs[1], "pj2": pjs[2],
            "wgt": wgt, "xn": xnat.reshape(128, 9 * XNA),
            "cscal": cscal,
        })

    nc = _build_nc(a_coef, b_coef, s)

    from concourse.bass_utils import run_bass_kernel_spmd
    res = run_bass_kernel_spmd(nc, in_maps, core_ids=list(range(NCORES)))
    pt_sum = 0.0
    ll_sum = 0.0
    for r in res.results:
        o = np.asarray(r["out"], np.float64)
        pt_sum += o[0:128].sum()
        ll_sum += o[128:256].sum()
    # zero rows 125..127 of xn contribute |0 - s| = s each, XNA per row
    n_phantom = NCORES * 3 * XNA
    loss = W_LOSS * (np.sqrt(ESCALE) * pt_sum + ll_sum
                     - n_phantom * s) / M_TOTAL
    return np.float32(loss)


# revision 18
# speedup vs baseline: 1.2196x; 1.2196x over previous
"""Bundle-adjustment loss kernel for 8 Trainium2 NeuronCores.

Data-parallel over the image axis M: each core processes exactly
12500 = 4*3125 images (no padding).

Device layout v2: partition dim = (quarter, camera) = 4*32 = 128 (all
partitions busy), free dim = (point, m') = 3*3125 = 9375.  The camera
transform R@P+t is done on the PE as 9 small matmuls per 512-block
(weights [13,128] encode delta(q,q')*R[c,i,j] plus a ones-row for t),
which also broadcasts the per-image point planes across cameras --- so
the DMA ships the compact [13,3125] point planes (0.24 MB) instead of
32x-replicated ones (7.4 MB).  The distortion polynomial + pixel
residual chain runs on wide [128, 3125] point-slices, emitted
slice-major with a fixed DVE/Pool/Act stage assignment (the "full"
row of ASSIGN below) tuned by interleaved A/B measurement: engine
handoffs inside a chain are expensive on this NRT, in-order engine
streams overlap only across slice boundaries, and DVE's 2x/4x modes
do not show up in practice.  Per-(q,c) camera scalars ride in scalar
slots of tensor_scalar / scalar_tensor_tensor / activation, so no
constant planes exist.  Observations arrive with the principal point
pre-subtracted (host) and the per-image losses accumulate on-chip via
activation(Sqrt, accum).  ~160-220 us/pass per core measured via the
reps-slope method (see test.py); v1 of this kernel was ~520 us.
"""

import numpy as np

M_TOTAL = 100000
C = 32
NCORES = 8
MCORE = 12500        # images per core
Q = 4                # image quarters -> partition = (q, c)
MQ = MCORE // Q      # 3125 images per quarter (free dim per point)
W = 3 * MQ           # 9375 total free width
BLK = 512            # matmul moving-dim block (one PSUM bank of f32)
KDIM = 13            # matmul contraction: 12 (q,j) rows + ones row
W_LOSS = 0.01        # LINE_W = LEN_W = REPROJ_W
ESCALE = 64.0        # e = (du^2+dv^2)/ESCALE; host multiplies by sqrt(ESCALE)
XNA = 100            # line/len images per partition row (125 rows used)

_NC_CACHE = {}


def _apply_tile_patch():
    """This walrus build rejects Tile's kernel-tail drain carrying every
    semaphore wait on one instruction ("Too many sync wait commands").
    Emit one wait_ge per live semaphore instead."""
    from concourse import tile

    if getattr(tile.TileContext, "_ba_drain_patched", False):
        return

    def _drain_and_barrier(self, tick_clock, wait_clock):
        nc = self.nc
        ticks = list(tick_clock.global_clock)
        allocated = wait_clock.sems.allocated()
        for key, sem in allocated.items():
            t = ticks[int(key)]
            if t > 0:
                nc.sync.wait_ge(sem, t)
        nc.sync.drain()
        nc.all_engine_barrier()
        assert self.sems is not None
        popped = nc._tile_sem_poison_stack.pop()
        assert popped is self._sem_poison
        nc.clear_and_free_semaphores(list(self.sems.allocated().values()))
        nc.all_engine_barrier()

    tile.TileContext._drain_and_barrier = _drain_and_barrier
    tile.TileContext._ba_drain_patched = True


def _spill_excess_waits(nc, cap=1):
    """This walrus build's ISA structs accept very few sync-wait slots
    per compute instruction. Spill waits beyond `cap` onto InstNoOp
    carriers inserted just before the instruction on the same engine."""
    import concourse.mybir as mybir
    import bass_rust

    fragile = {
        "InstTensorScalarPtr", "InstTensorScalar", "InstActivation",
        "InstReciprocal", "InstTensorReduce", "InstMatmult",
        "InstTensorCopy", "InstTensorTensor", "InstLdweights",
        "InstMemset", "InstIota", "InstTensorTensorReduce", "InstPool",
        "InstDMACopy", "InstDMA", "InstDmaTransposeAnt",
    }
    n_nop = 0
    for bb in nc.m.functions[0].blocks:
        il = bb.instructions
        out_list = []
        for inst in il:
            si = inst.sync_info
            if (si is not None and type(inst).__name__ in fragile
                    and len(si.on_wait) > cap):
                waits = list(si.on_wait)
                keep, spill = waits[:cap], waits[cap:]
                for wv in spill:
                    nop = mybir.InstNoOp(name=f"ba_waitnop_{n_nop}")
                    n_nop += 1
                    nop.engine = inst.engine
                    nop.sync_info = bass_rust.SyncInfo(
                        on_wait=[wv], on_update=[])
                    out_list.append(nop)
                inst.sync_info = bass_rust.SyncInfo(
                    on_wait=keep, on_update=list(si.on_update))
            out_list.append(inst)
        if len(out_list) != len(il):
            bb.instructions = out_list
    return n_nop


def _ap_key(arg):
    try:
        return str(arg)
    except Exception:
        return repr(arg)


def _dedup_ldweights(nc):
    """Drop InstLdweights whose payload equals the previous ldweights in
    the same block (PE array state is unchanged by intervening matmuls).
    Non-empty sync moves onto an InstNoOp carrier on the same engine."""
    import concourse.mybir as mybir

    n_drop = 0
    for bb in nc.m.functions[0].blocks:
        last = None
        keep = []
        for inst in bb.instructions:
            if type(inst).__name__ == "InstLdweights":
                key = tuple(_ap_key(a) for a in inst.ins)
                if last is not None and key == last:
                    si = inst.sync_info
                    if si is not None and (si.on_wait or si.on_update):
                        nop = mybir.InstNoOp(name=f"ldw_drop_{n_drop}")
                        nop.engine = inst.engine
                        nop.sync_info = si
                        keep.append(nop)
                    n_drop += 1
                    continue
                last = key
            keep.append(inst)
        if n_drop:
            bb.instructions = keep
    return n_drop


def _build_nc(a_coef, b_coef, s_len, reps=1, variant="full"):
    """Build the SPMD Bass module (same program on all 8 cores)."""
    key = (a_coef, b_coef, s_len, reps, variant)
    if key in _NC_CACHE:
        return _NC_CACHE[key]
    import concourse.bass as bass
    import concourse.mybir as mybir
    from concourse import tile

    _apply_tile_patch()
    F32 = mybir.dt.float32
    F16 = mybir.dt.float16
    F8 = mybir.dt.float8e4
    ALU = mybir.AluOpType
    ACT = mybir.ActivationFunctionType

    nc = bass.Bass(trn_type="TRN2")
    # inputs (per core)
    obs_u = nc.declare_dram_parameter("obs_u", [128, W], F16, isOutput=False)
    obs_v = nc.declare_dram_parameter("obs_v", [128, W], F16, isOutput=False)
    maskf = nc.declare_dram_parameter("maskf", [128, W], F16, isOutput=False)
    pj = [nc.declare_dram_parameter(f"pj{p}", [KDIM, MQ], F16,
                                    isOutput=False) for p in range(3)]
    wgt = nc.declare_dram_parameter("wgt", [KDIM, 384], F16, isOutput=False)
    xn = nc.declare_dram_parameter("xn", [128, 9 * XNA], F16, isOutput=False)
    # per-(q,c)-partition camera scalars, one column each:
    # 0:k1 1:k2 2:k3 3:2p1 4:2p2 5:fx*p2 6:fy*p1 7:fx 8:fy 9:-s(len bias)
    cscal = nc.declare_dram_parameter("cscal", [128, 12], F32, isOutput=False)
    out = nc.declare_dram_parameter("out", [256], F32, isOutput=True)

    # matmul m'-blocks: pairs of 512 -> one [128,1024] psum tile + 53 tail
    full_pairs = [(0, 1024), (1024, 1024), (2048, 1024)]
    tail = (3072, MQ - 3072)   # 53

    flags = set(variant.split("_"))
    nwin = 3
    for f in flags:
        if f and f[0] == "w" and f[1:].isdigit():
            nwin = int(f[1:])
    use_pm = "pm" in flags
    psa_bufs = 6 if use_pm else 3

    with tile.TileContext(nc) as tc:
        with tc.tile_pool(name="sb", bufs=1) as sb, \
             tc.tile_pool(name="psA", space="PSUM", bufs=psa_bufs) as ppA, \
             tc.tile_pool(name="psB", space="PSUM", bufs=2) as ppB:
            # --- resident tiles ---
            cs_t = sb.tile([128, 12], F32, tag="cs", name="cs_t")
            w_t = sb.tile([KDIM, 384], F16, tag="wg", name="w_t")
            pj_t = [sb.tile([KDIM, MQ], F16, tag=f"pj{p}", name=f"pj{p}_t")
                    for p in range(3)]
            ou_t = sb.tile([128, W], F16, tag="ou", name="ou_t")
            ov_t = sb.tile([128, W], F16, tag="ov", name="ov_t")
            mk_t = sb.tile([128, W], F16, tag="mk", name="mk_t")
            T0 = sb.tile([128, W], F16, tag="T0", name="T0")
            T1 = sb.tile([128, W], F16, tag="T1", name="T1")
            T2 = sb.tile([128, W], F16, tag="T2", name="T2")
            T3 = sb.tile([128, W], F16, tag="T3", name="T3")
            T4 = sb.tile([128, W], F16, tag="T4", name="T4")
            # line/len working tiles
            xn_t = sb.tile([128, 9 * XNA], F16, tag="xn", name="xn_t")
            g_t = sb.tile([128, 3 * XNA], F16, tag="g", name="g_t")
            dc_t = sb.tile([128, 6 * XNA], F16, tag="dc", name="dc_t")
            sq_t = sb.tile([128, 6 * XNA], F16, tag="sq", name="sq_t")
            rd_t = sb.tile([128, 2 * XNA], F32, tag="rd", name="rd_t")
            rt_t = sb.tile([128, 2 * XNA], F16, tag="rt", name="rt_t")
            ln_t = sb.tile([128, XNA], F16, tag="ln", name="ln_t")
            cb_t = sb.tile([128, XNA], F16, tag="cb", name="cb_t")
            # stages
            pt_stage = sb.tile([128, 8], F32, tag="pts", name="pt_stage")
            ll_stage = sb.tile([128, 1], F32, tag="lls", name="ll_stage")
            ptred = sb.tile([128, 1], F32, tag="ptr", name="ptred")
            # camera-scalar column APs
            k1s = cs_t[:, 0:1]
            k2s = cs_t[:, 1:2]
            k3s = cs_t[:, 2:3]
            tp1s = cs_t[:, 3:4]
            tp2s = cs_t[:, 4:5]
            fxp2s = cs_t[:, 5:6]
            fyp1s = cs_t[:, 6:7]
            fxs = cs_t[:, 7:8]
            fys = cs_t[:, 8:9]
            negs = cs_t[:, 9:10]

            # prologue loads (outside reps)
            nc.sync.dma_start(cs_t[:], cscal[:])
            nc.sync.dma_start(w_t[:], wgt[:])

            for _rep in range(reps):
                nc.sync.dma_start(xn_t[:], xn[:])
                for p in range(3):
                    nc.sync.dma_start(pj_t[p][:], pj[p][:])

                if variant == "dmaonly":
                    nc.sync.dma_start(ou_t[:], obs_u[:])
                    nc.scalar.dma_start(ov_t[:], obs_v[:])
                    nc.scalar.dma_start(mk_t[:], maskf[:])
                    nc.vector.tensor_reduce(
                        pt_stage[:, 0:1], ou_t[:],
                        mybir.AxisListType.X, ALU.add)
                    nc.vector.tensor_reduce(
                        ll_stage[:], xn_t[:], mybir.AxisListType.X, ALU.add)
                    continue

                # ---- line/len losses ----
                xnv = xn_t[:].rearrange("p (a j) -> p a j", a=XNA)
                x0s, x1s, x2s = xnv[:, :, 0:3], xnv[:, :, 3:6], xnv[:, :, 6:9]
                gv = g_t[:].rearrange("p (a j) -> p a j", a=XNA)
                dcv = dc_t[:].rearrange("p (a k j) -> p a k j", a=XNA, k=2)
                nc.vector.scalar_tensor_tensor(
                    gv, x0s, a_coef, x1s, ALU.mult, ALU.subtract)
                nc.vector.scalar_tensor_tensor(
                    dcv[:, :, 0, :], x2s, b_coef, gv, ALU.mult, ALU.add)
                nc.vector.scalar_tensor_tensor(
                    dcv[:, :, 1, :], x0s, 1.0, x2s, ALU.mult, ALU.subtract)
                nc.vector.scalar_tensor_tensor(
                    sq_t[:], dc_t[:], 1.0, dc_t[:], ALU.mult, ALU.mult)
                nc.vector.tensor_reduce(
                    rd_t[:].rearrange("p (a k) -> p a k", k=2),
                    sq_t[:].rearrange("p (a k j) -> p a k j", a=XNA, k=2),
                    mybir.AxisListType.X, ALU.add)
                nc.scalar.activation(rt_t[:], rd_t[:], ACT.Sqrt)
                rtv = rt_t[:].rearrange("p (a k) -> p a k", k=2)
                nc.scalar.activation(ln_t[:], rtv[:, :, 1], ACT.Abs,
                                     bias=negs)
                nc.gpsimd.tensor_tensor(
                    cb_t[:], rtv[:, :, 0], ln_t[:], ALU.add)
                nc.vector.tensor_reduce(
                    ll_stage[:], cb_t[:], mybir.AxisListType.X, ALU.add)

                # ---- reprojection loss ----
                vr2 = "vr2" in flags
                me = "me" in flags

                # chain windows over the full free width
                wb = [W * k // nwin for k in range(nwin + 1)]
                wins = [slice(wb[k], wb[k + 1]) for k in range(nwin)]

                # obs/mask loads, split across the SP and Act queues
                for cols in wins:
                    nc.sync.dma_start(ou_t[:, cols], obs_u[:, cols])
                    nc.scalar.dma_start(ov_t[:, cols], obs_v[:, cols])
                    nc.scalar.dma_start(mk_t[:, cols], maskf[:, cols])

                # camera transform on PE: for each coord i,
                # T_i[(q,c), m'] = sum_j R[c,i,j] P[q,p,j,m'] + t[c,i]
                dst = (T0, T1, T2)
                if not use_pm:
                    for p in range(3):
                        c0 = p * MQ
                        for i in range(3):
                            wap = w_t[:, 128 * i:128 * (i + 1)]
                            for b0, bw in full_pairs:
                                ps = ppA.tile([128, 1024], F32, tag="psA",
                                              name=f"ps_{_rep}_{p}_{i}_{b0}")
                                nc.tensor.matmul(
                                    ps[:, 0:512], wap,
                                    pj_t[p][:, b0:b0 + 512],
                                    start=True, stop=True)
                                nc.tensor.matmul(
                                    ps[:, 512:1024], wap,
                                    pj_t[p][:, b0 + 512:b0 + 1024],
                                    start=True, stop=True)
                                nc.scalar.activation(
                                    dst[i][:, c0 + b0:c0 + b0 + bw],
                                    ps[:], ACT.Copy)
                            b0, bw = tail
                            ps = ppB.tile([128, 64], F32, tag="psB",
                                          name=f"pst_{_rep}_{p}_{i}")
                            nc.tensor.matmul(
                                ps[:, 0:bw], wap, pj_t[p][:, b0:b0 + bw],
                                start=True, stop=True)
                            nc.scalar.activation(
                                dst[i][:, c0 + b0:c0 + b0 + bw],
                                ps[:, 0:bw], ACT.Copy)
                else:
                    # psum-direct: recip reads z from PSUM; x0n/x1n consume
                    # x,y from PSUM on Pool; no PSUM->SBUF copies at all.
                    blocks = [(0, 512), (512, 512), (1024, 512),
                              (1536, 512), (2048, 512), (2560, 512),
                              (3072, MQ - 3072)]
                    for p in range(3):
                        c0 = p * MQ
                        for b0, bw in blocks:
                            pxyz = []
                            for i in range(3):
                                wap = w_t[:, 128 * i:128 * (i + 1)]
                                ps = ppA.tile([128, 512], F32, tag="psA",
                                              name=f"ps_{_rep}_{p}_{i}_{b0}")
                                nc.tensor.matmul(
                                    ps[:, 0:bw], wap,
                                    pj_t[p][:, b0:b0 + bw],
                                    start=True, stop=True)
                                pxyz.append(ps)
                            bc = slice(c0 + b0, c0 + b0 + bw)
                            with nc.allow_low_precision(
                                    reason="1/Z in f16: Z~5, rel err "
                                           "~5e-4 vs 2e-2 budget"):
                                nc.vector.reciprocal(
                                    T3[:, bc], pxyz[2][:, 0:bw])
                            nc.vector.scalar_tensor_tensor(  # x0n
                                T0[:, bc], pxyz[0][:, 0:bw], 1.0,
                                T3[:, bc], ALU.mult, ALU.mult)
                            nc.vector.scalar_tensor_tensor(  # x1n
                                T1[:, bc], pxyz[1][:, 0:bw], 1.0,
                                T3[:, bc], ALU.mult, ALU.mult)

                # distortion + residual chain per window (slice-major:
                # each window's chain is consecutive in every engine's
                # stream; engines overlap across window boundaries).
                # Engine assignment per stage ("V"=DVE, "P"=Pool, "A"=Act)
                # is a 22-char string; chain order is fixed by the math.
                ASSIGN = {
                    "full": "VPPVVPAVVVVVVVVVVAAPPA",
                    "cl":   "VPPVVVVVVVVVVVVVVAAPPA",
                    "clp":  "VPPPPVVVVVVVVVVVVAAPPA",
                    "cla":  "VPPVVPVVVVVVVVVVVAAPPA",
                    "cld":  "VPPVVPAVVVVVVVVVVVVPPA",
                    "clc":  "VPPVVPAVVVVVVVVVVVVVVA",
                }
                akey = next((f for f in flags if f in ASSIGN), "full")
                asg = ASSIGN[akey]

                def _stt(e, dst, in0, sc, in1, op0, op1):
                    if e == "P":
                        assert (isinstance(sc, float) and sc == 1.0)
                        nc.gpsimd.tensor_tensor(dst, in0, in1, op1)
                    else:
                        nc.vector.scalar_tensor_tensor(
                            dst, in0, sc, in1, op0, op1)

                for wi, cols in enumerate(wins):
                    t0 = T0[:, cols]
                    t1 = T1[:, cols]
                    t2 = T2[:, cols]
                    t3 = T3[:, cols]
                    t4 = T4[:, cols]
                    ou = ou_t[:, cols]
                    ov = ov_t[:, cols]
                    mk = mk_t[:, cols]
                    if not use_pm:
                        with nc.allow_low_precision(
                                reason="1/Z in f16: Z~5, rel err ~5e-4 "
                                       "vs 2e-2 budget"):
                            nc.vector.reciprocal(t3, t2)            # 0
                        _stt(asg[1], t0, t0, 1.0, t3,
                             ALU.mult, ALU.mult)                    # 1 x0n
                        _stt(asg[2], t1, t1, 1.0, t3,
                             ALU.mult, ALU.mult)                    # 2 x1n
                    _stt(asg[3], t2, t0, 1.0, t0,
                         ALU.mult, ALU.mult)                        # 3 x0n^2
                    _stt(asg[4], t3, t1, 1.0, t1,
                         ALU.mult, ALU.mult)                        # 4 x1n^2
                    _stt(asg[5], t2, t2, 1.0, t3,
                         ALU.mult, ALU.add)                         # 5 r2
                    if asg[6] == "A":                               # 6 k3r2+k2
                        nc.scalar.activation(t3, t2, ACT.Identity,
                                             bias=k2s, scale=k3s)
                    else:
                        nc.vector.tensor_scalar(
                            t3, t2, k3s, k2s, ALU.mult, ALU.add)
                    _stt(asg[7], t3, t3, 1.0, t2,
                         ALU.mult, ALU.mult)                        # 7 *r2
                    nc.vector.scalar_tensor_tensor(                 # 8 +k1,*r2
                        t3, t3, k1s, t2, ALU.add, ALU.mult)
                    nc.vector.scalar_tensor_tensor(                 # 9 tan0
                        t4, t1, tp1s, t3, ALU.mult, ALU.add)
                    nc.vector.scalar_tensor_tensor(                 # 10 ra
                        t4, t0, tp2s, t4, ALU.mult, ALU.add)
                    nc.vector.scalar_tensor_tensor(                 # 11 mu
                        t3, t4, 1.0, t0, ALU.add, ALU.mult)
                    nc.vector.scalar_tensor_tensor(                 # 12 mv
                        t0, t4, 1.0, t1, ALU.add, ALU.mult)
                    nc.vector.scalar_tensor_tensor(                 # 13
                        t1, t3, fxs, ou, ALU.mult, ALU.subtract)
                    nc.vector.scalar_tensor_tensor(                 # 14 du
                        t1, t2, fxp2s, t1, ALU.mult, ALU.add)
                    nc.vector.scalar_tensor_tensor(                 # 15
                        t4, t0, fys, ov, ALU.mult, ALU.subtract)
                    nc.vector.scalar_tensor_tensor(                 # 16 dv
                        t4, t2, fyp1s, t4, ALU.mult, ALU.add)
                    if asg[17] == "A":                              # 17
                        nc.scalar.activation(t0, t1, ACT.Square,
                                             scale=0.125)
                    else:
                        nc.vector.scalar_tensor_tensor(
                            t0, t1, 1.0 / ESCALE, t1, ALU.mult, ALU.mult)
                    if asg[18] == "A":                              # 18
                        nc.scalar.activation(t3, t4, ACT.Square,
                                             scale=0.125)
                    else:
                        nc.vector.scalar_tensor_tensor(
                            t3, t4, 1.0 / ESCALE, t4, ALU.mult, ALU.mult)
                    _stt(asg[19], t0, t0, 1.0, t3,
                         ALU.mult, ALU.add)                         # 19 e
                    _stt(asg[20], t0, t0, 1.0, mk,
                         ALU.mult, ALU.mult)                        # 20 mask
                    nc.scalar.activation(t4, t0, ACT.Sqrt,          # 21 sum
                                         accum_out=pt_stage[:, wi:wi + 1])

            # ---- epilogue ----
            nc.vector.tensor_reduce(ptred[:], pt_stage[:, 0:nwin],
                                    mybir.AxisListType.X, ALU.add)
            nc.sync.dma_start(out[0:128], ptred[:])
            nc.sync.dma_start(out[128:256], ll_stage[:])

    _dedup_ldweights(nc)
    _spill_excess_waits(nc)
    _NC_CACHE[key] = nc
    return nc


def kernel(pole, pole_3ds, pole_2ds, mask, K, dist, R, t):
    pole = np.asarray(pole, np.float32)
    pole_3ds = np.asarray(pole_3ds, np.float32)
    pole_2ds = np.asarray(pole_2ds, np.float32)
    mask = np.asarray(mask)
    K = np.asarray(K, np.float32)
    dist = np.asarray(dist, np.float32)
    R = np.asarray(R, np.float32)
    t = np.asarray(t, np.float32)

    s = float(pole[0] + pole[1])
    a_coef = float(pole[1] / s)   # coefficient of X0 in exp_p1
    b_coef = float(pole[0] / s)   # coefficient of X2

    def rep(v):  # [C] -> [(q,c)] = tile over quarters
        return np.tile(v.astype(np.float32), Q)

    cscal = np.zeros((128, 12), np.float32)
    cscal[:, 0] = rep(dist[:, 0])                 # k1
    cscal[:, 1] = rep(dist[:, 1])                 # k2
    cscal[:, 2] = rep(dist[:, 4])                 # k3
    cscal[:, 3] = rep(2.0 * dist[:, 2])           # 2*p1
    cscal[:, 4] = rep(2.0 * dist[:, 3])           # 2*p2
    cscal[:, 5] = rep(K[:, 0, 0] * dist[:, 3])    # fx*p2
    cscal[:, 6] = rep(K[:, 1, 1] * dist[:, 2])    # fy*p1
    cscal[:, 7] = rep(K[:, 0, 0])                 # fx
    cscal[:, 8] = rep(K[:, 1, 1])                 # fy
    cscal[:, 9] = -s                              # len-loss bias

    # matmul weights: wgt[q*3+j, i*128 + q*32+c] = R[c,i,j];
    # ones-row 12 -> t[c,i]
    wgt = np.zeros((KDIM, 384), np.float16)
    for i in range(3):
        for q in range(Q):
            blockc = i * 128 + q * 32
            for j in range(3):
                wgt[q * 3 + j, blockc:blockc + 32] = R[:, i, j]
            wgt[12, blockc:blockc + 32] = t[:, i]

    u0_c = K[:, 0, 2]   # [C]
    v0_c = K[:, 1, 2]

    in_maps = []
    for core in range(NCORES):
        m0 = core * MCORE
        # obs planes [128, 9375]: row=q*32+c, col=p*MQ+m'
        p2 = pole_2ds[m0:m0 + MCORE].reshape(Q, MQ, C, 3, 2)
        ou = np.ascontiguousarray(
            (p2[..., 0] - u0_c[None, None, :, None])
            .transpose(0, 2, 3, 1)          # [q, c, p, m']
            .reshape(128, W).astype(np.float16))
        ov = np.ascontiguousarray(
            (p2[..., 1] - v0_c[None, None, :, None])
            .transpose(0, 2, 3, 1)
            .reshape(128, W).astype(np.float16))
        mkq = mask[m0:m0 + MCORE].reshape(Q, MQ, C).transpose(0, 2, 1)
        mk = np.ascontiguousarray(
            np.broadcast_to(mkq[:, :, None, :], (Q, C, 3, MQ))
            .reshape(128, W).astype(np.float16))
        # compact point planes pj[p][q*3+j, m'] = P[m, p, j]; row 12 = 1
        p3 = pole_3ds[m0:m0 + MCORE].reshape(Q, MQ, 3, 3)
        pjs = []
        for p in range(3):
            pl = np.zeros((KDIM, MQ), np.float16)
            pl[0:12] = p3[:, :, p, :].transpose(0, 2, 1).reshape(12, MQ)
            pl[12] = 1.0
            pjs.append(pl)
        # line/len layout: image m_local = r*XNA + a -> xn[r, 9a:9a+9]
        xnat = np.zeros((128, XNA, 9), np.float16)
        xnat[:125] = pole_3ds[m0:m0 + MCORE].reshape(125, XNA, 9)
        in_maps.append({
            "obs_u": ou, "obs_v": ov, "maskf": mk,
            "pj0": pjs[0], "pj1": pjs[1], "pj2": pjs[2],
            "wgt": wgt, "xn": xnat.reshape(128, 9 * XNA),
            "cscal": cscal,
        })

    nc = _build_nc(a_coef, b_coef, s)

    from concourse.bass_utils import run_bass_kernel_spmd
    res = run_bass_kernel_spmd(nc, in_maps, core_ids=list(range(NCORES)))
    pt_sum = 0.0
    ll_sum = 0.0
    for r in res.results:
        o = np.asarray(r["out"], np.float64)
        pt_sum += o[0:128].sum()
        ll_sum += o[128:256].sum()
    # zero rows 125..127 of xn contribute |0 - s| = s each, XNA per row
    n_phantom = NCORES * 3 * XNA
    loss = W_LOSS * (np.sqrt(ESCALE) * pt_sum + ll_sum
                     - n_phantom * s) / M_TOTAL
    return np.float32(loss)
